# revision 15
# baseline (speedup 1.0000x reference)
"""Trainium2 Bass kernel for nn_Decoder (GRU decoder step with ragged attention).

Strategy (8 NeuronCores):
- Attention: data-parallel over batch (4 batches/core). encoder_output is
  streamed from HBM exactly once per core ([128,1024] w-tiles stay SBUF-resident
  per batch): scores pass (PE broadcast-add -> ACT tanh -> DVE fused
  mul+reduce), then context pass (PE matvec accumulation, unnormalized
  softmax weights; normalization folded into the PSUM->SBUF copy).
- GRU: output(gate)-sharded across cores (128 h-units per core), with host
  pre-transposed/sliced weights and biases folded in as extra matmul K-rows.
  Cross-core exchange via AllGather(context), AllGather(h0), AllReduce(logits).
- Host does index-only prep (argmax/embedding gather, masks, transposes,
  slicing) and output assembly (concat, attn normalization).
All matmuls use float32r views (full-rate PE) except tiny ones.
"""
import numpy as np

import concourse.bass as bass
import concourse.mybir as mybir
import concourse.tile as tile
from concourse.bacc import Bacc
from concourse.bass_utils import run_bass_kernel_spmd

F32 = mybir.dt.float32
F32R = mybir.dt.float32r
AX = mybir.AxisListType
OP = mybir.AluOpType
ACTF = mybir.ActivationFunctionType

B, W, H, L, E, V = 32, 2048, 1024, 2, 60, 100
NCORES = 8
BPC = B // NCORES          # batches per core
HS = H // NCORES           # gate-dim slice per core
G3 = 3 * HS                # 384 gate columns per core
KE = E + 1                 # emb chunk rows (emb + ones row)


def _mm(nc, out, lhsT, rhs, start, stop=False, skip_gc=False):
    """matmul out += lhsT.T @ rhs with N-dim split to <=512 (f32r inputs)."""
    n = rhs.shape[-1]
    for n0 in range(0, n, 512):
        n1 = min(n0 + 512, n)
        nc.tensor.matmul(
            out[:, n0:n1],
            lhsT,
            rhs[:, n0:n1],
            start=start,
            stop=stop,
            skip_group_check=skip_gc,
        )


def build(width=W):
    WC = width // 128
    nc = Bacc(num_devices=NCORES)

    # ---- I/O -------------------------------------------------------------
    enc = nc.declare_dram_parameter("enc", [width, BPC, H], F32, False)
    hmT = nc.declare_dram_parameter("hmT", [L, H, BPC], F32, False)  # hidT own cols
    h_sl = nc.declare_dram_parameter("h_sl", [L, B, HS], F32, False)
    hT0e = nc.declare_dram_parameter("hT0e", [H + 1, B], F32, False)
    hT1e = nc.declare_dram_parameter("hT1e", [H + 1, B], F32, False)
    whpT = nc.declare_dram_parameter("whpT", [H, H], F32, False)  # 0.5*W_hp.T
    bhp = nc.declare_dram_parameter("bhp", [1, H], F32, False)
    woa = nc.declare_dram_parameter("woa", [128, H], F32, False)  # replicated
    maskt = nc.declare_dram_parameter("maskt", [BPC, 128, WC], F32, False)
    ident = nc.declare_dram_parameter("ident", [128, 128], F32, False)
    onesr = nc.declare_dram_parameter("onesr", [1, 128], F32, False)
    onesc = nc.declare_dram_parameter("onesc", [128, 1], F32, False)
    embTe = nc.declare_dram_parameter("embTe", [KE, B], F32, False)
    wg0 = nc.declare_dram_parameter("wg0", [H + KE, G3], F32, False)
    wh0 = nc.declare_dram_parameter("wh0", [H + 1, G3], F32, False)
    wg1 = nc.declare_dram_parameter("wg1", [H + 1, G3], F32, False)
    wh1 = nc.declare_dram_parameter("wh1", [H + 1, G3], F32, False)
    woT = nc.declare_dram_parameter("woT", [HS, V], F32, False)
    bout = nc.declare_dram_parameter("bout", [1, V], F32, False)

    p_out = nc.declare_dram_parameter("p_out", [BPC, 128, WC], F32, True)
    h0_out = nc.declare_dram_parameter("h0_out", [B, HS], F32, True)
    h1_out = nc.declare_dram_parameter("h1_out", [B, HS], F32, True)
    probs = nc.declare_dram_parameter("probs", [B, V], F32, True)

    KC = H // 128  # 8 k-chunks of 128

    with tile.TileContext(nc) as tc:
        with (
            tc.tile_pool(name="encp", bufs=min(2 * WC + 2, 17)) as encp,
            tc.tile_pool(name="cst", bufs=1) as cst,
            tc.tile_pool(name="whp", bufs=2) as whpp,
            tc.tile_pool(name="res", bufs=3) as resp,
            tc.tile_pool(name="prod", bufs=2) as prodp,
            tc.tile_pool(name="sc", bufs=4) as scp,
            tc.tile_pool(name="gw", bufs=2) as gwp,
            tc.tile_pool(name="sm", bufs=3) as smp,
            tc.tile_pool(name="psA", bufs=2, space="PSUM") as psA,
            tc.tile_pool(name="psB", bufs=1, space="PSUM") as psB,
            tc.tile_pool(name="psC", bufs=2, space="PSUM") as psC,
            tc.tile_pool(name="dram", bufs=8, space="DRAM") as drp,
        ):
            # ---- constants -------------------------------------------------
            id_t = cst.tile([128, 128], F32R, tag="id")
            nc.sync.dma_start(id_t[:], ident[:].bitcast(F32R))
            or_t = cst.tile([1, 128], F32R, tag="or")
            nc.sync.dma_start(or_t[:], onesr[:].bitcast(F32R))
            oc_t = cst.tile([128, 1], F32R, tag="oc")
            nc.sync.dma_start(oc_t[:], onesc[:].bitcast(F32R))
            woa_t = cst.tile([128, H], F32, tag="woa")
            nc.sync.dma_start(woa_t[:], woa[:])
            bhp_t = cst.tile([1, H], F32R, tag="bhp")
            nc.sync.dma_start(bhp_t[:], bhp[:].bitcast(F32R))

            # ---- ha = (hid0+hid1) @ (0.5*W_hp.T) + b_hp  (own batches) -----
            ha_ps = psB.tile([BPC, H], F32, tag="bigps")
            for kc in range(KC):
                hm2 = smp.tile([128, 2 * BPC], F32, tag="hm2")
                nc.sync.dma_start(hm2[:, :BPC], hmT[0, kc * 128:(kc + 1) * 128, :])
                nc.sync.dma_start(hm2[:, BPC:], hmT[1, kc * 128:(kc + 1) * 128, :])
                hmsum32 = smp.tile([128, BPC], F32, tag="hmsum32")
                hmsum = smp.tile([128, BPC], F32R, tag="hmsum")
                whp_t = whpp.tile([128, H], F32R, tag="whp")
                nc.sync.dma_start(whp_t[:], whpT[kc * 128:(kc + 1) * 128, :].bitcast(F32R))
                nc.vector.tensor_tensor(out=hmsum32[:], in0=hm2[:, :BPC],
                                        in1=hm2[:, BPC:], op=OP.add)
                nc.sync.dma_start(hmsum[:], hmsum32[:].bitcast(F32R))
                _mm(nc, ha_ps, hmsum[:], whp_t[:], start=(kc == 0))
            _mm(nc, ha_ps, or_t[:, :BPC], bhp_t[:], start=False, stop=True)
            ha_sb4 = cst.tile([BPC, H], F32, tag="ha4")
            nc.vector.tensor_copy(ha_sb4[:], ha_ps[:])
            ha_rows = []
            for b4 in range(BPC):
                har = cst.tile([1, H], F32R, tag=f"har{b4}")
                nc.sync.dma_start(har[:], ha_sb4[b4:b4 + 1, :].bitcast(F32R))
                ha_rows.append(har)

            # ---- attention -------------------------------------------------
            ctx_loc = drp.tile([BPC, H], F32, tag="ctx_loc")
            enc_tiles = {}
            p_tiles = []
            for b4 in range(BPC):
                scores = scp.tile([128, WC], F32, tag="scores")
                for wc in range(WC):
                    et = encp.tile([128, H], F32R, tag="enc")
                    nc.sync.dma_start(et[:], enc[wc * 128:(wc + 1) * 128, b4, :].bitcast(F32R))
                    enc_tiles[(b4, wc)] = et
                    ps_r = psA.tile([128, H], F32, tag="res")
                    _mm(nc, ps_r, or_t[:], ha_rows[b4][:], start=True)
                    _mm(nc, ps_r, id_t[:], et[:], start=False, stop=True)
                    rt = resp.tile([128, H], F32, tag="res")
                    nc.scalar.activation(rt[:], ps_r[:], ACTF.Tanh)
                    pt = prodp.tile([128, H], F32, tag="prod")
                    nc.vector.scalar_tensor_tensor(
                        out=pt[:],
                        in0=rt[:],
                        scalar=1.0,
                        in1=woa_t[:],
                        op0=OP.mult,
                        op1=OP.mult,
                        accum_out=scores[:, wc:wc + 1],
                    )
                # p = exp(scores) * mask
                ep = scp.tile([128, WC], F32, tag="ep")
                nc.scalar.activation(ep[:], scores[:], ACTF.Exp)
                mt = scp.tile([128, WC], F32, tag="mt")
                nc.sync.dma_start(mt[:], maskt[b4, :, :])
                pw32 = scp.tile([128, WC], F32, tag="pw32")
                nc.vector.tensor_tensor(out=pw32[:], in0=ep[:], in1=mt[:], op=OP.mult)
                pw = scp.tile([128, WC], F32R, tag="pw")
                nc.sync.dma_start(pw[:], pw32[:].bitcast(F32R))
                p_tiles.append(pw)
                nc.sync.dma_start(p_out[b4, :, :], pw32[:])
                # context accumulation (unnormalized) + denominator
                ctx_ps = psB.tile([1, H], F32, tag="bigps")
                den_ps = psC.tile([1, 1], F32, tag="smallps")
                for wc in range(WC):
                    et = enc_tiles[(b4, wc)]
                    _mm(nc, ctx_ps, pw[:, wc:wc + 1], et[:], start=(wc == 0), stop=(wc == WC - 1))
                    nc.tensor.matmul(
                        den_ps[:],
                        pw[:, wc:wc + 1].bitcast(F32),
                        oc_t[:].bitcast(F32),
                        start=(wc == 0),
                        stop=(wc == WC - 1),
                    )
                rcp = smp.tile([1, 1], F32, tag="rcp")
                nc.vector.reciprocal(rcp[:], den_ps[:])
                ctx_sb = smp.tile([1, H], F32, tag="ctx_sb")
                nc.vector.tensor_scalar(
                    out=ctx_sb[:], in0=ctx_ps[:], scalar1=rcp[:], scalar2=None,
                    op0=OP.mult,
                )
                nc.sync.dma_start(ctx_loc[b4:b4 + 1, :], ctx_sb[:])

            # ---- GRU weights prefetch -------------------------------------
            wg0_t = [gwp.tile([128, G3], F32R, tag="gw", bufs=16, name=f"wg0t{i}") for i in range(KC)]
            for kc in range(KC):
                nc.sync.dma_start(wg0_t[kc][:], wg0[kc * 128:(kc + 1) * 128, :].bitcast(F32R))
            wg0_e = gwp.tile([KE, G3], F32R, tag="gwe")
            nc.sync.dma_start(wg0_e[:], wg0[H:H + KE, :].bitcast(F32R))
            wh0_t = [gwp.tile([128, G3], F32R, tag="gw", bufs=16, name=f"wh0t{i}") for i in range(KC)]
            for kc in range(KC):
                nc.sync.dma_start(wh0_t[kc][:], wh0[kc * 128:(kc + 1) * 128, :].bitcast(F32R))
            wh0_b = gwp.tile([1, G3], F32R, tag="gwb", bufs=3)
            nc.sync.dma_start(wh0_b[:], wh0[H:H + 1, :].bitcast(F32R))
            wg1_t = [gwp.tile([128, G3], F32R, tag="gw", bufs=16, name=f"wg1t{i}") for i in range(KC)]
            for kc in range(KC):
                nc.sync.dma_start(wg1_t[kc][:], wg1[kc * 128:(kc + 1) * 128, :].bitcast(F32R))
            wg1_b = gwp.tile([1, G3], F32R, tag="gwb", bufs=3)
            nc.sync.dma_start(wg1_b[:], wg1[H:H + 1, :].bitcast(F32R))
            wh1_t = [gwp.tile([128, G3], F32R, tag="gw", bufs=16, name=f"wh1t{i}") for i in range(KC)]
            for kc in range(KC):
                nc.sync.dma_start(wh1_t[kc][:], wh1[kc * 128:(kc + 1) * 128, :].bitcast(F32R))
            wh1_b = gwp.tile([1, G3], F32R, tag="gwb", bufs=3)
            nc.sync.dma_start(wh1_b[:], wh1[H:H + 1, :].bitcast(F32R))
            embT_t = gwp.tile([KE, B], F32R, tag="embT")
            nc.sync.dma_start(embT_t[:], embTe[:].bitcast(F32R))
            hT0_t = [gwp.tile([128, B], F32R, tag="hT", bufs=32, name=f"hT0t{i}") for i in range(KC)]
            for kc in range(KC):
                nc.sync.dma_start(hT0_t[kc][:], hT0e[kc * 128:(kc + 1) * 128, :].bitcast(F32R))
            hT0_b = gwp.tile([1, B], F32R, tag="hTb", bufs=2)
            nc.sync.dma_start(hT0_b[:], hT0e[H:H + 1, :].bitcast(F32R))
            hT1_t = [gwp.tile([128, B], F32R, tag="hT", bufs=32, name=f"hT1t{i}") for i in range(KC)]
            for kc in range(KC):
                nc.sync.dma_start(hT1_t[kc][:], hT1e[kc * 128:(kc + 1) * 128, :].bitcast(F32R))
            hT1_b = gwp.tile([1, B], F32R, tag="hTb", bufs=2)
            nc.sync.dma_start(hT1_b[:], hT1e[H:H + 1, :].bitcast(F32R))
            woT_t = gwp.tile([HS, V], F32R, tag="woT")
            nc.sync.dma_start(woT_t[:], woT[:].bitcast(F32R))
            bout_t = gwp.tile([1, V], F32R, tag="bout")
            nc.sync.dma_start(bout_t[:], bout[:].bitcast(F32R))
            hp_t = [gwp.tile([B, HS], F32, tag="hp", bufs=2, name=f"hpt{i}") for i in range(L)]
            for l in range(L):
                nc.sync.dma_start(hp_t[l][:], h_sl[l, :, :])

            # ---- AllGather context ----------------------------------------
            ctx_all = drp.tile([B, H], F32, tag="ctx_all")
            nc.gpsimd.collective_compute(
                "AllGather", OP.bypass,
                replica_groups=[list(range(NCORES))],
                ins=[ctx_loc.opt()], outs=[ctx_all.opt()],
            )
            ctxT_t = [gwp.tile([128, B], F32R, tag="hT", bufs=32, name=f"ctxTt{i}") for i in range(KC)]
            for kc in range(KC):
                nc.sync.dma_start(
                    ctxT_t[kc][:],
                    ctx_all[:, kc * 128:(kc + 1) * 128].rearrange("a b -> b a").bitcast(F32R),
                )

            def gru_layer(xT_tiles, xT_extra, wg_tiles, wg_extra, hT_tiles, hT_b,
                          wh_tiles, wh_b, h_prev):
                """One GRU layer for this core's 128-unit gate slice.
                Returns h_new [B, HS] sbuf tile.
                psum_g layout: [:,0:256]=r|z (gi+gh), [:,256:384]=inn, [:,384:512]=hn
                """
                g_ps = psC.tile([B, 4 * HS], F32, tag="smallps")
                # one PSUM bank: exactly one start (first mm) and one stop
                # (last mm); first write to each untouched region inside the
                # started bank behaves as overwrite (pending-zero semantics).
                for kc, xt in enumerate(xT_tiles):
                    _mm(nc, g_ps[:, 0:2 * HS], xt[:], wg_tiles[kc][:, 0:2 * HS],
                        start=(kc == 0))
                    _mm(nc, g_ps[:, 2 * HS:3 * HS], xt[:],
                        wg_tiles[kc][:, 2 * HS:3 * HS], start=False)
                _mm(nc, g_ps[:, 0:2 * HS], xT_extra[:], wg_extra[:, 0:2 * HS],
                    start=False)
                _mm(nc, g_ps[:, 2 * HS:3 * HS], xT_extra[:],
                    wg_extra[:, 2 * HS:3 * HS], start=False)
                for kc, ht in enumerate(hT_tiles):
                    _mm(nc, g_ps[:, 0:2 * HS], ht[:], wh_tiles[kc][:, 0:2 * HS],
                        start=False)
                    _mm(nc, g_ps[:, 3 * HS:4 * HS], ht[:],
                        wh_tiles[kc][:, 2 * HS:3 * HS], start=False)
                _mm(nc, g_ps[:, 0:2 * HS], hT_b[:], wh_b[:, 0:2 * HS], start=False)
                _mm(nc, g_ps[:, 3 * HS:4 * HS], hT_b[:], wh_b[:, 2 * HS:3 * HS],
                    start=False, stop=True)
                # gates
                trz = smp.tile([B, 2 * HS], F32, tag="trz")
                nc.scalar.activation(trz[:], g_ps[:, 0:2 * HS], ACTF.Tanh, scale=0.5)
                rz = smp.tile([B, 2 * HS], F32, tag="rz")
                nc.vector.tensor_scalar(out=rz[:], in0=trz[:], scalar1=0.5,
                                        scalar2=0.5, op0=OP.mult, op1=OP.add)
                t1 = smp.tile([B, HS], F32, tag="t1")
                nc.vector.tensor_tensor(out=t1[:], in0=rz[:, 0:HS],
                                        in1=g_ps[:, 3 * HS:4 * HS], op=OP.mult)
                t2 = smp.tile([B, HS], F32, tag="t2")
                nc.vector.tensor_tensor(out=t2[:], in0=t1[:],
                                        in1=g_ps[:, 2 * HS:3 * HS], op=OP.add)
                n_sb = smp.tile([B, HS], F32, tag="n")
                nc.scalar.activation(n_sb[:], t2[:], ACTF.Tanh)
                d = smp.tile([B, HS], F32, tag="d")
                nc.vector.tensor_tensor(out=d[:], in0=h_prev[:], in1=n_sb[:],
                                        op=OP.subtract)
                zd = smp.tile([B, HS], F32, tag="zd")
                nc.vector.tensor_tensor(out=zd[:], in0=rz[:, HS:2 * HS], in1=d[:],
                                        op=OP.mult)
                h_new = smp.tile([B, HS], F32, tag="hnew")
                nc.vector.tensor_tensor(out=h_new[:], in0=n_sb[:], in1=zd[:],
                                        op=OP.add)
                return h_new

            # ---- layer 0 ---------------------------------------------------
            h0_sb = gru_layer(ctxT_t, embT_t, wg0_t, wg0_e,
                              hT0_t, hT0_b, wh0_t, wh0_b, hp_t[0])
            nc.sync.dma_start(h0_out[:], h0_sb[:])
            h0_loc = drp.tile([B, HS], F32, tag="h0_loc")
            nc.sync.dma_start(h0_loc[:], h0_sb[:])
            h0_all = drp.tile([NCORES * B, HS], F32, tag="h0_all")
            nc.gpsimd.collective_compute(
                "AllGather", OP.bypass,
                replica_groups=[list(range(NCORES))],
                ins=[h0_loc.opt()], outs=[h0_all.opt()],
            )
            h0T_t = [gwp.tile([128, B], F32R, tag="hT", bufs=32, name=f"h0Tt{i}") for i in range(KC)]
            for kc in range(KC):
                nc.sync.dma_start(
                    h0T_t[kc][:],
                    h0_all[kc * B:(kc + 1) * B, :].rearrange("a b -> b a").bitcast(F32R),
                )

            # ---- layer 1 ---------------------------------------------------
            h1_sb = gru_layer(h0T_t, or_t[:, :B], wg1_t, wg1_b,
                              hT1_t, hT1_b, wh1_t, wh1_b, hp_t[1])
            nc.sync.dma_start(h1_out[:], h1_sb[:])

            # ---- logits: h1 @ W_out.T (K-sharded) + AllReduce -------------
            t_ps = psC.tile([128, B], F32, tag="smallps")
            nc.tensor.transpose(t_ps[:], h1_sb[:], id_t[:B, :B].bitcast(F32))
            h1T32 = smp.tile([HS, B], F32, tag="h1T32")
            nc.vector.tensor_copy(h1T32[:], t_ps[:HS, :])
            h1T = smp.tile([HS, B], F32R, tag="h1T")
            nc.sync.dma_start(h1T[:], h1T32[:].bitcast(F32R))
            lg_ps = psC.tile([B, V], F32, tag="smallps")
            _mm(nc, lg_ps, h1T[:], woT_t[:], start=True)
            _mm(nc, lg_ps, or_t[:, :B], bout_t[:], start=False, stop=True)
            lg_sb = smp.tile([B, V], F32, tag="lg")
            nc.vector.tensor_copy(lg_sb[:], lg_ps[:])
            lg_loc = drp.tile([B, V], F32, tag="lg_loc")
            nc.sync.dma_start(lg_loc[:], lg_sb[:])
            lg_all = drp.tile([B, V], F32, tag="lg_all")
            nc.gpsimd.collective_compute(
                "AllReduce", OP.add,
                replica_groups=[list(range(NCORES))],
                ins=[lg_loc.opt()], outs=[lg_all.opt()],
            )
            lgin = smp.tile([B, V], F32, tag="lgin")
            nc.sync.dma_start(lgin[:], lg_all[:])
            e_sb = smp.tile([B, V], F32, tag="esb")
            nc.scalar.activation(e_sb[:], lgin[:], ACTF.Exp)
            s_t = smp.tile([B, 1], F32, tag="st")
            nc.vector.tensor_reduce(s_t[:], e_sb[:], AX.X, OP.add)
            rs_t = smp.tile([B, 1], F32, tag="rst")
            nc.vector.reciprocal(rs_t[:], s_t[:])
            pr_sb = smp.tile([B, V], F32, tag="prsb")
            nc.vector.tensor_scalar(out=pr_sb[:], in0=e_sb[:], scalar1=rs_t[:],
                                    scalar2=None, op0=OP.mult)
            nc.sync.dma_start(probs[:], pr_sb[:])

    nc.compile()
    return nc


_CACHE = {}


def _get_nc(width):
    if width not in _CACHE:
        _CACHE[width] = build(width)
    return _CACHE[width]


def make_in_maps(in_char, hidden, encoder_output, src_len, embedding, W_hp, b_hp,
                 W_oa, b_oa, Wih0, Whh0, bih0, bhh0, Wih1, Whh1, bih1, bhh1,
                 W_out, b_out):
    """Host-side preprocessing: build the per-core input maps."""
    width = encoder_output.shape[0]
    WC = width // 128
    f32 = np.float32

    # enc_len exactly as the reference (float32 arithmetic)
    scale = f32(width) / src_len[0].astype(f32)
    enc_len = np.floor(src_len.astype(f32) * scale + f32(0.999)).astype(np.int32)

    # argmax/embedding (index-only)
    top1 = np.argmax(in_char, axis=1)
    embT = np.ascontiguousarray(embedding[top1].T)  # [E, B]
    embTe = np.concatenate([embT, np.ones((1, B), f32)], axis=0)  # [E+1, B]

    hT0 = np.ascontiguousarray(hidden[0].T)  # [H, B]
    hT1 = np.ascontiguousarray(hidden[1].T)
    ones_row = np.ones((1, B), f32)
    hT0e = np.concatenate([hT0, ones_row], axis=0)
    hT1e = np.concatenate([hT1, ones_row], axis=0)
    hmT_full = np.stack([hT0, hT1], axis=0)  # [L, H, B] (mean folded into whpT)

    whpT = np.ascontiguousarray(W_hp.T) * f32(0.5)
    woa_rep = np.repeat(W_oa.astype(f32), 128, axis=0)  # [128, H]
    identity = np.eye(128, dtype=f32)
    onesr = np.ones((1, 128), f32)
    onesc = np.ones((128, 1), f32)

    # [128, WC] w-index grid: w = wc*128 + r
    wgrid = np.arange(width, dtype=np.int32).reshape(WC, 128).T  # [128, WC]

    def gate_slice_T(X, bvec, c, reorder_emb=False):
        sl = slice(HS * c, HS * (c + 1))
        rows = np.concatenate([X[sl], X[H + sl.start:H + sl.stop],
                               X[2 * H + sl.start:2 * H + sl.stop]], axis=0)
        rhs = np.ascontiguousarray(rows.T).astype(f32)  # [K, 384]
        bias = np.concatenate([bvec[sl], bvec[H + sl.start:H + sl.stop],
                               bvec[2 * H + sl.start:2 * H + sl.stop]])[None, :]
        if reorder_emb:
            # K rows [0:E]=emb, [E:E+H]=ctx  ->  [ctx(H); emb(E); bias]
            rhs = np.concatenate([rhs[E:E + H], rhs[0:E], bias.astype(f32)], axis=0)
        else:
            rhs = np.concatenate([rhs, bias.astype(f32)], axis=0)
        return np.ascontiguousarray(rhs)

    in_maps = []
    for c in range(NCORES):
        bs = slice(BPC * c, BPC * (c + 1))
        hsl = slice(HS * c, HS * (c + 1))
        mask = (wgrid[None, :, :] < enc_len[bs][:, None, None]).astype(f32)
        m = {
            "enc": np.ascontiguousarray(encoder_output[:, bs, :]),
            "hmT": np.ascontiguousarray(hmT_full[:, :, bs]),
            "h_sl": np.ascontiguousarray(hidden[:, :, hsl]),
            "hT0e": hT0e, "hT1e": hT1e,
            "whpT": whpT, "bhp": b_hp[None, :].astype(f32),
            "woa": woa_rep, "maskt": mask,
            "ident": identity, "onesr": onesr, "onesc": onesc,
            "embTe": embTe,
            "wg0": gate_slice_T(Wih0, bih0, c, reorder_emb=True),
            "wh0": gate_slice_T(Whh0, bhh0, c),
            "wg1": gate_slice_T(Wih1, bih1, c),
            "wh1": gate_slice_T(Whh1, bhh1, c),
            "woT": np.ascontiguousarray(W_out[:, hsl].T),
            "bout": (b_out / f32(NCORES))[None, :].astype(f32),
        }
        in_maps.append({k: np.ascontiguousarray(v, dtype=f32) for k, v in m.items()})
    return in_maps


def assemble(results, width):
    """Host-side output assembly from per-core results."""
    attn = np.zeros((B, width), np.float32)
    for c in range(NCORES):
        p = results[c]["p_out"]  # [BPC, 128, WC]
        for b4 in range(BPC):
            row = p[b4].T.reshape(width)  # w = wc*128 + r
            s = row.sum(dtype=np.float32)
            attn[BPC * c + b4] = row / s
    h0 = np.concatenate([results[c]["h0_out"] for c in range(NCORES)], axis=1)
    h1 = np.concatenate([results[c]["h1_out"] for c in range(NCORES)], axis=1)
    latest_hidden = np.stack([h0, h1], axis=0)
    output = results[0]["probs"]
    return output, latest_hidden, attn


def kernel(**inputs):
    inputs = {k: np.asarray(v) for k, v in inputs.items()}
    width = inputs["encoder_output"].shape[0]
    nc = _get_nc(width)
    in_maps = make_in_maps(**inputs)
    res = run_bass_kernel_spmd(nc, in_maps, list(range(NCORES)))
    return assemble(res.results, width)
